# revision 29
# baseline (speedup 1.0000x reference)
"""BottomRightPool (2D cummax) Trainium2 Bass kernel.

pool[b,c,i,j] = max(x[b,c,:i+1,:j+1])  ==  cummax over H, then over W.

Key identity: pool rows are non-decreasing along w, so
    pool[i, :] = scan_j ( state = max(state, x[i, j], pool[i-1, j]) )
and cummax_w(pool[i-1, :]) == pool[i-1, :], so ONE scan instruction per row
(data stream = x row i, second stream = pool row i-1) performs BOTH cummax
passes. The scan itself is a registered Layer-2 custom DVE op (see
_make_cummax_op): out = scan(MAX, max(src0, src1)).

Perf notes (all numbers measured on this HW via loop-slope):
  - All HBM traffic is bf16: tolerance is 2e-2 and bf16 rounding is ~2e-3
    (max() is exact in bf16, so error == input rounding). Halves DMA vs
    f32: 33.55 MB/core -> ~104 us at the measured 322 GB/s 8-core
    concurrent rate. That DMA floor is the kernel's bound.
  - Scans must run on DVE (TensorScalarPtr/TensorTensor are illegal on the
    Pool engine for NC v3; Pool ops that do compile run ~20 us/instr). The
    stock tensor_tensor_scan costs ~2.2-2.4 ns/elem (two-ALU-stage state
    feedback forces a per-element bubble); the custom single-stage-feedback
    op runs ~2.09 ns/elem isolated and pipelines much better under DMA.
  - 4 lanes (one per 128-slice chunk) round-robin per row so adjacent DVE
    scans come from independent chains.
  - HB=16 with deep tile pools (5 generations) pipelines DMA fully under
    the scans; in-DMAs issue on SP, out-DMAs alternate Act/Pool so no
    single sequencer's DGE time (~0.6 us/DMA) becomes a serial tail.
    (HB=32 re-measured worse, 126.6 vs 120 us, with the custom op too.)
  - Net ~117-120 us/core, ~13 us over the DMA roofline (pipeline
    fill/drain + residual SBUF port contention).
"""

import numpy as np


def _make_cummax_op():
    """Register (once) a Layer-2 custom DVE op: fused cummax scan.

    The stock tensor_tensor_scan routes its state through two ALU stages
    (op0 then op1), which forces a per-element feedback bubble (~2.2-2.4
    ns/elem measured). A max-scan is associative, so
    state' = max(state, max(x, prev_pool)) keeps the state in a single
    stage's CURR_ALU_OUT temporal feedback: measured ~2.09 ns/elem
    (267 ns vs 330 ns per 128-elem row scan), exact on HW.
    """
    import re

    import concourse.dve_ops as dve_ops
    from concourse.dve_ops import DveOp
    from concourse.dve_spec import Spec, Src0, Src1, scan, maxx
    from concourse.dve_uop import AluOp

    name = "CUMMAX_FUSED_ANT"
    for o in dve_ops.OPS:
        if o.name == name:
            return o

    def _ref(in0, in1, s0, s1, imm2):
        return np.maximum.accumulate(
            np.maximum(in0, in1), axis=-1
        ).astype(np.float32)

    spec = Spec(body=scan(AluOp.MAX, maxx(Src0, Src1)), reference=_ref)
    opcode = max(dve_ops._SUB_OPCODE_FOR_NAME.values()) + 1
    assert opcode < 0x20
    dve_ops._SUB_OPCODE_FOR_NAME[name] = opcode
    try:
        DveOp(name, spec, subdim=False, uops_sha={}).compile("v3")
        raise AssertionError("compile must raise to reveal the uops sha")
    except ValueError as e:
        sha = re.search(r"v3: (\w+) ", str(e)).group(1)
    op = DveOp(name, spec, subdim=False, uops_sha={"v3": sha})
    dve_ops.OPS.append(op)
    return op


N_CORES = 8
B, C, H, W = 16, 256, 128, 128
S = B * C                    # 4096 independent (b,c) slices
SPC = S // N_CORES           # 512 slices per core
CHUNK = 128                  # slices per tile (partition dim)
HB = 16                      # rows per h-block tile
NEG = -3.0e38

# Engine per lane (lane = slice chunk): "v" = DVE, "p" = GPSIMD/Pool.
# (Pool rejected: TensorScalarPtr is not a legal Pool opcode on NC v3.)
LANE_ENGINES = ("v", "v", "v", "v")


def _build_nc(repeat=None):
    """Build the per-core Bass program. repeat=None emits the plain kernel;
    repeat=R wraps the whole workload in a hardware For_i loop (benchmarking
    only — output is just rewritten R times)."""
    import concourse.mybir as mybir
    import concourse.tile as tile
    from concourse import bacc

    cummax_op = _make_cummax_op()
    nc = bacc.Bacc(None, target_bir_lowering=False)
    DT = mybir.dt.bfloat16
    xd = nc.dram_tensor("x", [SPC, H, W], DT, kind="ExternalInput")
    od = nc.dram_tensor("out", [SPC, H, W], DT, kind="ExternalOutput")
    MAX = mybir.AluOpType.max

    n_lanes = SPC // CHUNK
    assert n_lanes == len(LANE_ENGINES)

    with tile.TileContext(nc) as tc:
        with tc.tile_pool(name="ina", bufs=5 * n_lanes) as pa, tc.tile_pool(
            name="outb", bufs=5 * n_lanes + 2
        ) as pb:

            def body():
                engines = [
                    nc.vector if e == "v" else nc.gpsimd for e in LANE_ENGINES
                ]
                prev = [None] * n_lanes  # pool row above current block
                for hb in range(H // HB):
                    h0 = hb * HB
                    tiles = []
                    for lane in range(n_lanes):
                        s0 = lane * CHUNK
                        A = pa.tile([CHUNK, HB * W], DT)
                        Bt = pb.tile([CHUNK, HB * W], DT)
                        nc.sync.dma_start(
                            out=A[:],
                            in_=xd[s0 : s0 + CHUNK, h0 : h0 + HB].rearrange(
                                "s h w -> s (h w)"
                            ),
                        )
                        tiles.append((A, Bt))
                    for r in range(HB):
                        row = slice(r * W, (r + 1) * W)
                        for lane, (A, Bt) in enumerate(tiles):
                            if r == 0 and prev[lane] is None:
                                data1 = A[:, row]
                            elif r == 0:
                                data1 = prev[lane]
                            else:
                                data1 = Bt[:, (r - 1) * W : r * W]
                            nc.vector._custom_dve(
                                cummax_op,
                                out=Bt[:, row],
                                in0=A[:, row],
                                in1=data1,
                            )
                    for lane, (A, Bt) in enumerate(tiles):
                        s0 = lane * CHUNK
                        prev[lane] = Bt[:, (HB - 1) * W : HB * W]
                        out_eng = nc.scalar if lane % 2 == 0 else nc.gpsimd
                        out_eng.dma_start(
                            out=od[s0 : s0 + CHUNK, h0 : h0 + HB].rearrange(
                                "s h w -> s (h w)"
                            ),
                            in_=Bt[:],
                        )

            if repeat is None:
                body()
            else:
                with tc.For_i(0, repeat, 1):
                    body()
    nc.compile()
    return nc


def make_runner(nc, donate=True):
    """Compile once; return run(in_maps) plus the raw jitted callable.

    Mirrors concourse.bass2jax.run_bass_via_pjrt's multi-core path but keeps
    the jitted executable so repeated calls don't re-trace/re-compile.
    donate=False keeps passed device buffers alive so the bench can call the
    executable repeatedly with device-resident args (no host transfers).
    """
    import jax
    import concourse.mybir as mybir
    from jax.sharding import Mesh, PartitionSpec
    from jax.experimental.shard_map import shard_map
    from concourse.bass2jax import (
        _bass_exec_p,
        install_neuronx_cc_hook,
        partition_id_tensor,
    )

    install_neuronx_cc_hook()
    assert nc.dbg_addr is None
    partition_name = nc.partition_id_tensor.name if nc.partition_id_tensor else None

    in_names, out_names, out_avals, zero_outs = [], [], [], []
    for alloc in nc.m.functions[0].allocations:
        if not isinstance(alloc, mybir.MemoryLocationSet):
            continue
        name = alloc.memorylocations[0].name
        if alloc.kind == "ExternalInput":
            if name == partition_name:
                continue
            in_names.append(name)
        elif alloc.kind == "ExternalOutput":
            out_names.append(name)
            shape = tuple(alloc.tensor_shape)
            dtype = mybir.dt.np(alloc.dtype)
            out_avals.append(jax.core.ShapedArray(shape, dtype))
            zero_outs.append(np.zeros(shape, dtype))
    n_params = len(in_names)
    n_outs = len(out_avals)
    all_in_names = in_names + out_names
    if partition_name is not None:
        all_in_names = all_in_names + [partition_name]
    donate_idx = tuple(range(n_params, n_params + n_outs)) if donate else ()

    def _body(*args):
        operands = list(args)
        if partition_name is not None:
            operands.append(partition_id_tensor())
        outs = _bass_exec_p.bind(
            *operands,
            out_avals=tuple(out_avals),
            in_names=tuple(all_in_names),
            out_names=tuple(out_names),
            lowering_input_output_aliases=(),
            sim_require_finite=True,
            sim_require_nnan=True,
            nc=nc,
        )
        return tuple(outs)

    devices = jax.devices()[:N_CORES]
    mesh = Mesh(np.asarray(devices), ("core",))
    sharded = jax.jit(
        shard_map(
            _body,
            mesh=mesh,
            in_specs=(PartitionSpec("core"),) * (n_params + n_outs),
            out_specs=(PartitionSpec("core"),) * n_outs,
            check_rep=False,
        ),
        donate_argnums=donate_idx,
        keep_unused=True,
    )

    def make_args(in_maps):
        concat_in = [
            np.concatenate([np.asarray(m[name]) for m in in_maps], axis=0)
            for name in in_names
        ]
        concat_zeros = [
            np.zeros((N_CORES * z.shape[0], *z.shape[1:]), z.dtype)
            for z in zero_outs
        ]
        return concat_in + concat_zeros

    def run(in_maps):
        out_arrs = sharded(*make_args(in_maps))
        return [
            {
                name: np.asarray(out_arrs[i]).reshape(
                    N_CORES, *out_avals[i].shape
                )[c]
                for i, name in enumerate(out_names)
            }
            for c in range(N_CORES)
        ]

    return run, sharded, make_args


def _in_maps(xf: np.ndarray):
    """Shard the [S, H, W] input into per-core input dicts (bf16 on device)."""
    import ml_dtypes

    xb = np.asarray(xf, dtype=ml_dtypes.bfloat16)
    return [{"x": xb[k * SPC : (k + 1) * SPC]} for k in range(N_CORES)]


def _run(x: np.ndarray, trace: bool = False):
    """Returns (full_output, exec_time_ns_or_None)."""
    nc = _build_nc()
    run, _, _ = make_runner(nc)
    xf = np.ascontiguousarray(x, dtype=np.float32).reshape(S, H, W)
    in_maps = _in_maps(xf)
    results = run(in_maps)
    out = np.concatenate([r["out"] for r in results], axis=0)
    return out.astype(np.float32).reshape(B, C, H, W), None


def kernel(x: np.ndarray) -> np.ndarray:
    return _run(x)[0]


# revision 30
# speedup vs baseline: 1.0193x; 1.0193x over previous
"""BottomRightPool (2D cummax) Trainium2 Bass kernel.

pool[b,c,i,j] = max(x[b,c,:i+1,:j+1])  ==  cummax over H, then over W.

Key identity: pool rows are non-decreasing along w, so
    pool[i, :] = scan_j ( state = max(state, x[i, j], pool[i-1, j]) )
and cummax_w(pool[i-1, :]) == pool[i-1, :], so ONE scan instruction per row
(data stream = x row i, second stream = pool row i-1) performs BOTH cummax
passes. The scan itself is a registered Layer-2 custom DVE op (see
_make_cummax_op): out = scan(MAX, max(src0, src1)).

Perf notes (all numbers measured on this HW via loop-slope):
  - All HBM traffic is bf16: tolerance is 2e-2 and bf16 rounding is ~2e-3
    (max() is exact in bf16, so error == input rounding). Halves DMA vs
    f32: 33.55 MB/core -> ~104 us at the measured 322 GB/s 8-core
    concurrent rate. That DMA floor is the kernel's bound.
  - Scans must run on DVE (TensorScalarPtr/TensorTensor are illegal on the
    Pool engine for NC v3; Pool ops that do compile run ~20 us/instr). The
    stock tensor_tensor_scan costs ~2.2-2.4 ns/elem (two-ALU-stage state
    feedback forces a per-element bubble); the custom single-stage-feedback
    op runs ~2.09 ns/elem isolated and pipelines much better under DMA.
  - 4 lanes (one per 128-slice chunk) round-robin per row so adjacent DVE
    scans come from independent chains.
  - HB=16 with deep tile pools (5 generations) pipelines DMA fully under
    the scans; in-DMAs issue on SP, out-DMAs alternate Act/Pool so no
    single sequencer's DGE time (~0.6 us/DMA) becomes a serial tail.
    (HB=32 re-measured worse, 126.6 vs 120 us, with the custom op too.)
  - Net ~117-120 us/core, ~13 us over the DMA roofline (pipeline
    fill/drain + residual SBUF port contention).
"""

import numpy as np


def _make_cummax_op():
    """Register (once) a Layer-2 custom DVE op: fused cummax scan.

    The stock tensor_tensor_scan routes its state through two ALU stages
    (op0 then op1), which forces a per-element feedback bubble (~2.2-2.4
    ns/elem measured). A max-scan is associative, so
    state' = max(state, max(x, prev_pool)) keeps the state in a single
    stage's CURR_ALU_OUT temporal feedback: measured ~2.09 ns/elem
    (267 ns vs 330 ns per 128-elem row scan), exact on HW.
    """
    import re

    import concourse.dve_ops as dve_ops
    from concourse.dve_ops import DveOp
    from concourse.dve_spec import Spec, Src0, Src1, scan, maxx
    from concourse.dve_uop import AluOp

    name = "CUMMAX_FUSED_ANT"
    for o in dve_ops.OPS:
        if o.name == name:
            return o

    def _ref(in0, in1, s0, s1, imm2):
        return np.maximum.accumulate(
            np.maximum(in0, in1), axis=-1
        ).astype(np.float32)

    spec = Spec(body=scan(AluOp.MAX, maxx(Src0, Src1)), reference=_ref)
    opcode = max(dve_ops._SUB_OPCODE_FOR_NAME.values()) + 1
    assert opcode < 0x20
    dve_ops._SUB_OPCODE_FOR_NAME[name] = opcode
    try:
        DveOp(name, spec, subdim=False, uops_sha={}).compile("v3")
        raise AssertionError("compile must raise to reveal the uops sha")
    except ValueError as e:
        sha = re.search(r"v3: (\w+) ", str(e)).group(1)
    op = DveOp(name, spec, subdim=False, uops_sha={"v3": sha})
    dve_ops.OPS.append(op)
    return op


N_CORES = 8
B, C, H, W = 16, 256, 128, 128
S = B * C                    # 4096 independent (b,c) slices
SPC = S // N_CORES           # 512 slices per core
CHUNK = 128                  # slices per tile (partition dim)
HB = 16                      # rows per h-block tile
NEG = -3.0e38

# Engine per lane (lane = slice chunk): "v" = DVE, "p" = GPSIMD/Pool.
# (Pool rejected: TensorScalarPtr is not a legal Pool opcode on NC v3.)
LANE_ENGINES = ("v", "v", "v", "v")


def _build_nc(repeat=None):
    """Build the per-core Bass program. repeat=None emits the plain kernel;
    repeat=R wraps the whole workload in a hardware For_i loop (benchmarking
    only — output is just rewritten R times)."""
    import concourse.mybir as mybir
    import concourse.tile as tile
    from concourse import bacc

    cummax_op = _make_cummax_op()
    nc = bacc.Bacc(None, target_bir_lowering=False)
    DT = mybir.dt.bfloat16
    xd = nc.dram_tensor("x", [SPC, H, W], DT, kind="ExternalInput")
    od = nc.dram_tensor("out", [SPC, H, W], DT, kind="ExternalOutput")
    MAX = mybir.AluOpType.max

    n_lanes = SPC // CHUNK
    assert n_lanes == len(LANE_ENGINES)

    with tile.TileContext(nc) as tc:
        with tc.tile_pool(name="ina", bufs=5 * n_lanes) as pa, tc.tile_pool(
            name="outb", bufs=5 * n_lanes + 2
        ) as pb:

            def body():
                # Variable h-block schedule: small first/last blocks shrink
                # the exposed pipeline fill (first in-DMA before scans can
                # start) and drain (last out-DMA after the last scan).
                blocks, h0acc = [], 0
                for nrows in (4, 8) + (16,) * 7 + (4,):
                    blocks.append((h0acc, nrows))
                    h0acc += nrows
                assert h0acc == H
                prev = [None] * n_lanes  # pool row above current block
                for h0, HBv in blocks:
                    tiles = []
                    for lane in range(n_lanes):
                        s0 = lane * CHUNK
                        A = pa.tile([CHUNK, HBv * W], DT)
                        Bt = pb.tile([CHUNK, HBv * W], DT)
                        nc.sync.dma_start(
                            out=A[:],
                            in_=xd[s0 : s0 + CHUNK, h0 : h0 + HBv].rearrange(
                                "s h w -> s (h w)"
                            ),
                        )
                        tiles.append((A, Bt))
                    for r in range(HBv):
                        row = slice(r * W, (r + 1) * W)
                        for lane, (A, Bt) in enumerate(tiles):
                            if r == 0 and prev[lane] is None:
                                data1 = A[:, row]
                            elif r == 0:
                                data1 = prev[lane]
                            else:
                                data1 = Bt[:, (r - 1) * W : r * W]
                            nc.vector._custom_dve(
                                cummax_op,
                                out=Bt[:, row],
                                in0=A[:, row],
                                in1=data1,
                            )
                    for lane, (A, Bt) in enumerate(tiles):
                        s0 = lane * CHUNK
                        prev[lane] = Bt[:, (HBv - 1) * W : HBv * W]
                        out_eng = nc.scalar if lane % 2 == 0 else nc.gpsimd
                        out_eng.dma_start(
                            out=od[s0 : s0 + CHUNK, h0 : h0 + HBv].rearrange(
                                "s h w -> s (h w)"
                            ),
                            in_=Bt[:],
                        )

            if repeat is None:
                body()
            else:
                with tc.For_i(0, repeat, 1):
                    body()
    nc.compile()
    return nc


def make_runner(nc, donate=True):
    """Compile once; return run(in_maps) plus the raw jitted callable.

    Mirrors concourse.bass2jax.run_bass_via_pjrt's multi-core path but keeps
    the jitted executable so repeated calls don't re-trace/re-compile.
    donate=False keeps passed device buffers alive so the bench can call the
    executable repeatedly with device-resident args (no host transfers).
    """
    import jax
    import concourse.mybir as mybir
    from jax.sharding import Mesh, PartitionSpec
    from jax.experimental.shard_map import shard_map
    from concourse.bass2jax import (
        _bass_exec_p,
        install_neuronx_cc_hook,
        partition_id_tensor,
    )

    install_neuronx_cc_hook()
    assert nc.dbg_addr is None
    partition_name = nc.partition_id_tensor.name if nc.partition_id_tensor else None

    in_names, out_names, out_avals, zero_outs = [], [], [], []
    for alloc in nc.m.functions[0].allocations:
        if not isinstance(alloc, mybir.MemoryLocationSet):
            continue
        name = alloc.memorylocations[0].name
        if alloc.kind == "ExternalInput":
            if name == partition_name:
                continue
            in_names.append(name)
        elif alloc.kind == "ExternalOutput":
            out_names.append(name)
            shape = tuple(alloc.tensor_shape)
            dtype = mybir.dt.np(alloc.dtype)
            out_avals.append(jax.core.ShapedArray(shape, dtype))
            zero_outs.append(np.zeros(shape, dtype))
    n_params = len(in_names)
    n_outs = len(out_avals)
    all_in_names = in_names + out_names
    if partition_name is not None:
        all_in_names = all_in_names + [partition_name]
    donate_idx = tuple(range(n_params, n_params + n_outs)) if donate else ()

    def _body(*args):
        operands = list(args)
        if partition_name is not None:
            operands.append(partition_id_tensor())
        outs = _bass_exec_p.bind(
            *operands,
            out_avals=tuple(out_avals),
            in_names=tuple(all_in_names),
            out_names=tuple(out_names),
            lowering_input_output_aliases=(),
            sim_require_finite=True,
            sim_require_nnan=True,
            nc=nc,
        )
        return tuple(outs)

    devices = jax.devices()[:N_CORES]
    mesh = Mesh(np.asarray(devices), ("core",))
    sharded = jax.jit(
        shard_map(
            _body,
            mesh=mesh,
            in_specs=(PartitionSpec("core"),) * (n_params + n_outs),
            out_specs=(PartitionSpec("core"),) * n_outs,
            check_rep=False,
        ),
        donate_argnums=donate_idx,
        keep_unused=True,
    )

    def make_args(in_maps):
        concat_in = [
            np.concatenate([np.asarray(m[name]) for m in in_maps], axis=0)
            for name in in_names
        ]
        concat_zeros = [
            np.zeros((N_CORES * z.shape[0], *z.shape[1:]), z.dtype)
            for z in zero_outs
        ]
        return concat_in + concat_zeros

    def run(in_maps):
        out_arrs = sharded(*make_args(in_maps))
        return [
            {
                name: np.asarray(out_arrs[i]).reshape(
                    N_CORES, *out_avals[i].shape
                )[c]
                for i, name in enumerate(out_names)
            }
            for c in range(N_CORES)
        ]

    return run, sharded, make_args


def _in_maps(xf: np.ndarray):
    """Shard the [S, H, W] input into per-core input dicts (bf16 on device)."""
    import ml_dtypes

    xb = np.asarray(xf, dtype=ml_dtypes.bfloat16)
    return [{"x": xb[k * SPC : (k + 1) * SPC]} for k in range(N_CORES)]


def _run(x: np.ndarray, trace: bool = False):
    """Returns (full_output, exec_time_ns_or_None)."""
    nc = _build_nc()
    run, _, _ = make_runner(nc)
    xf = np.ascontiguousarray(x, dtype=np.float32).reshape(S, H, W)
    in_maps = _in_maps(xf)
    results = run(in_maps)
    out = np.concatenate([r["out"] for r in results], axis=0)
    return out.astype(np.float32).reshape(B, C, H, W), None


def kernel(x: np.ndarray) -> np.ndarray:
    return _run(x)[0]


# revision 31
# speedup vs baseline: 1.0207x; 1.0014x over previous
"""BottomRightPool (2D cummax) Trainium2 Bass kernel.

pool[b,c,i,j] = max(x[b,c,:i+1,:j+1])  ==  cummax over H, then over W.

Key identity: pool rows are non-decreasing along w, so
    pool[i, :] = scan_j ( state = max(state, x[i, j], pool[i-1, j]) )
and cummax_w(pool[i-1, :]) == pool[i-1, :], so ONE scan instruction per row
(data stream = x row i, second stream = pool row i-1) performs BOTH cummax
passes. The scan itself is a registered Layer-2 custom DVE op (see
_make_cummax_op): out = scan(MAX, max(src0, src1)).

Perf notes (all numbers measured on this HW via loop-slope):
  - All HBM traffic is bf16: tolerance is 2e-2 and bf16 rounding is ~2e-3
    (max() is exact in bf16, so error == input rounding). Halves DMA vs
    f32: 33.55 MB/core -> ~104 us at the measured 322 GB/s 8-core
    concurrent rate. That DMA floor is the kernel's bound.
  - Scans must run on DVE (TensorScalarPtr/TensorTensor are illegal on the
    Pool engine for NC v3; Pool ops that do compile run ~20 us/instr). The
    stock tensor_tensor_scan costs ~2.2-2.4 ns/elem (two-ALU-stage state
    feedback forces a per-element bubble); the custom single-stage-feedback
    op runs ~2.09 ns/elem isolated and pipelines much better under DMA.
  - 4 lanes (one per 128-slice chunk) round-robin per row so adjacent DVE
    scans come from independent chains.
  - Variable h-block schedule (4,8,16x7,4 rows) with deep tile pools
    (~5 generations): small first/last blocks shrink the exposed pipeline
    fill/drain; in-DMAs issue on SP, out-DMAs alternate Act/Pool so no
    single sequencer's DGE time (~0.6 us/DMA) becomes a serial tail.
    (Fixed HB=32 re-measured worse, 126.6 vs 120 us, with the custom op.)
  - Net ~116-117 us/core, ~12 us over the DMA roofline (residual SBUF
    port contention between the scan streams and concurrent DMA).
"""

import numpy as np


def _make_cummax_op():
    """Register (once) a Layer-2 custom DVE op: fused cummax scan.

    The stock tensor_tensor_scan routes its state through two ALU stages
    (op0 then op1), which forces a per-element feedback bubble (~2.2-2.4
    ns/elem measured). A max-scan is associative, so
    state' = max(state, max(x, prev_pool)) keeps the state in a single
    stage's CURR_ALU_OUT temporal feedback: measured ~2.09 ns/elem
    (267 ns vs 330 ns per 128-elem row scan), exact on HW.
    """
    import re

    import concourse.dve_ops as dve_ops
    from concourse.dve_ops import DveOp
    from concourse.dve_spec import Spec, Src0, Src1, scan, maxx
    from concourse.dve_uop import AluOp

    name = "CUMMAX_FUSED_ANT"
    for o in dve_ops.OPS:
        if o.name == name:
            return o

    def _ref(in0, in1, s0, s1, imm2):
        return np.maximum.accumulate(
            np.maximum(in0, in1), axis=-1
        ).astype(np.float32)

    spec = Spec(body=scan(AluOp.MAX, maxx(Src0, Src1)), reference=_ref)
    opcode = max(dve_ops._SUB_OPCODE_FOR_NAME.values()) + 1
    assert opcode < 0x20
    dve_ops._SUB_OPCODE_FOR_NAME[name] = opcode
    try:
        DveOp(name, spec, subdim=False, uops_sha={}).compile("v3")
        raise AssertionError("compile must raise to reveal the uops sha")
    except ValueError as e:
        sha = re.search(r"v3: (\w+) ", str(e)).group(1)
    op = DveOp(name, spec, subdim=False, uops_sha={"v3": sha})
    dve_ops.OPS.append(op)
    return op


N_CORES = 8
B, C, H, W = 16, 256, 128, 128
S = B * C                    # 4096 independent (b,c) slices
SPC = S // N_CORES           # 512 slices per core
CHUNK = 128                  # slices per tile (partition dim)
HB = 16                      # rows per h-block tile
NEG = -3.0e38

# Engine per lane (lane = slice chunk): "v" = DVE, "p" = GPSIMD/Pool.
# (Pool rejected: TensorScalarPtr is not a legal Pool opcode on NC v3.)
LANE_ENGINES = ("v", "v", "v", "v")


def _build_nc(repeat=None):
    """Build the per-core Bass program. repeat=None emits the plain kernel;
    repeat=R wraps the whole workload in a hardware For_i loop (benchmarking
    only — output is just rewritten R times)."""
    import concourse.mybir as mybir
    import concourse.tile as tile
    from concourse import bacc

    cummax_op = _make_cummax_op()
    nc = bacc.Bacc(None, target_bir_lowering=False)
    DT = mybir.dt.bfloat16
    xd = nc.dram_tensor("x", [SPC, H, W], DT, kind="ExternalInput")
    od = nc.dram_tensor("out", [SPC, H, W], DT, kind="ExternalOutput")
    MAX = mybir.AluOpType.max

    n_lanes = SPC // CHUNK
    assert n_lanes == len(LANE_ENGINES)

    with tile.TileContext(nc) as tc:
        with tc.tile_pool(name="ina", bufs=5 * n_lanes) as pa, tc.tile_pool(
            name="outb", bufs=5 * n_lanes + 2
        ) as pb:

            def body():
                # Variable h-block schedule: small first/last blocks shrink
                # the exposed pipeline fill (first in-DMA before scans can
                # start) and drain (last out-DMA after the last scan).
                blocks, h0acc = [], 0
                for nrows in (4, 8) + (16,) * 7 + (4,):
                    blocks.append((h0acc, nrows))
                    h0acc += nrows
                assert h0acc == H
                prev = [None] * n_lanes  # pool row above current block
                for h0, HBv in blocks:
                    tiles = []
                    for lane in range(n_lanes):
                        s0 = lane * CHUNK
                        A = pa.tile([CHUNK, HBv * W], DT)
                        Bt = pb.tile([CHUNK, HBv * W], DT)
                        nc.sync.dma_start(
                            out=A[:],
                            in_=xd[s0 : s0 + CHUNK, h0 : h0 + HBv].rearrange(
                                "s h w -> s (h w)"
                            ),
                        )
                        tiles.append((A, Bt))
                    for r in range(HBv):
                        row = slice(r * W, (r + 1) * W)
                        for lane, (A, Bt) in enumerate(tiles):
                            if r == 0 and prev[lane] is None:
                                data1 = A[:, row]
                            elif r == 0:
                                data1 = prev[lane]
                            else:
                                data1 = Bt[:, (r - 1) * W : r * W]
                            nc.vector._custom_dve(
                                cummax_op,
                                out=Bt[:, row],
                                in0=A[:, row],
                                in1=data1,
                            )
                    for lane, (A, Bt) in enumerate(tiles):
                        s0 = lane * CHUNK
                        prev[lane] = Bt[:, (HBv - 1) * W : HBv * W]
                        out_eng = nc.scalar if lane % 2 == 0 else nc.gpsimd
                        out_eng.dma_start(
                            out=od[s0 : s0 + CHUNK, h0 : h0 + HBv].rearrange(
                                "s h w -> s (h w)"
                            ),
                            in_=Bt[:],
                        )

            if repeat is None:
                body()
            else:
                with tc.For_i(0, repeat, 1):
                    body()
    nc.compile()
    return nc


def make_runner(nc, donate=True):
    """Compile once; return run(in_maps) plus the raw jitted callable.

    Mirrors concourse.bass2jax.run_bass_via_pjrt's multi-core path but keeps
    the jitted executable so repeated calls don't re-trace/re-compile.
    donate=False keeps passed device buffers alive so the bench can call the
    executable repeatedly with device-resident args (no host transfers).
    """
    import jax
    import concourse.mybir as mybir
    from jax.sharding import Mesh, PartitionSpec
    from jax.experimental.shard_map import shard_map
    from concourse.bass2jax import (
        _bass_exec_p,
        install_neuronx_cc_hook,
        partition_id_tensor,
    )

    install_neuronx_cc_hook()
    assert nc.dbg_addr is None
    partition_name = nc.partition_id_tensor.name if nc.partition_id_tensor else None

    in_names, out_names, out_avals, zero_outs = [], [], [], []
    for alloc in nc.m.functions[0].allocations:
        if not isinstance(alloc, mybir.MemoryLocationSet):
            continue
        name = alloc.memorylocations[0].name
        if alloc.kind == "ExternalInput":
            if name == partition_name:
                continue
            in_names.append(name)
        elif alloc.kind == "ExternalOutput":
            out_names.append(name)
            shape = tuple(alloc.tensor_shape)
            dtype = mybir.dt.np(alloc.dtype)
            out_avals.append(jax.core.ShapedArray(shape, dtype))
            zero_outs.append(np.zeros(shape, dtype))
    n_params = len(in_names)
    n_outs = len(out_avals)
    all_in_names = in_names + out_names
    if partition_name is not None:
        all_in_names = all_in_names + [partition_name]
    donate_idx = tuple(range(n_params, n_params + n_outs)) if donate else ()

    def _body(*args):
        operands = list(args)
        if partition_name is not None:
            operands.append(partition_id_tensor())
        outs = _bass_exec_p.bind(
            *operands,
            out_avals=tuple(out_avals),
            in_names=tuple(all_in_names),
            out_names=tuple(out_names),
            lowering_input_output_aliases=(),
            sim_require_finite=True,
            sim_require_nnan=True,
            nc=nc,
        )
        return tuple(outs)

    devices = jax.devices()[:N_CORES]
    mesh = Mesh(np.asarray(devices), ("core",))
    sharded = jax.jit(
        shard_map(
            _body,
            mesh=mesh,
            in_specs=(PartitionSpec("core"),) * (n_params + n_outs),
            out_specs=(PartitionSpec("core"),) * n_outs,
            check_rep=False,
        ),
        donate_argnums=donate_idx,
        keep_unused=True,
    )

    def make_args(in_maps):
        concat_in = [
            np.concatenate([np.asarray(m[name]) for m in in_maps], axis=0)
            for name in in_names
        ]
        concat_zeros = [
            np.zeros((N_CORES * z.shape[0], *z.shape[1:]), z.dtype)
            for z in zero_outs
        ]
        return concat_in + concat_zeros

    def run(in_maps):
        out_arrs = sharded(*make_args(in_maps))
        return [
            {
                name: np.asarray(out_arrs[i]).reshape(
                    N_CORES, *out_avals[i].shape
                )[c]
                for i, name in enumerate(out_names)
            }
            for c in range(N_CORES)
        ]

    return run, sharded, make_args


def _in_maps(xf: np.ndarray):
    """Shard the [S, H, W] input into per-core input dicts (bf16 on device)."""
    import ml_dtypes

    xb = np.asarray(xf, dtype=ml_dtypes.bfloat16)
    return [{"x": xb[k * SPC : (k + 1) * SPC]} for k in range(N_CORES)]


def _run(x: np.ndarray, trace: bool = False):
    """Returns (full_output, exec_time_ns_or_None)."""
    nc = _build_nc()
    run, _, _ = make_runner(nc)
    xf = np.ascontiguousarray(x, dtype=np.float32).reshape(S, H, W)
    in_maps = _in_maps(xf)
    results = run(in_maps)
    out = np.concatenate([r["out"] for r in results], axis=0)
    return out.astype(np.float32).reshape(B, C, H, W), None


def kernel(x: np.ndarray) -> np.ndarray:
    return _run(x)[0]
